# revision 4
# baseline (speedup 1.0000x reference)
"""Trainium2 Bass kernel for nn_BBoxDecoder (HyperNetwork -> per-sample CoordinateNet).

Computation (fp32):
    h1   = relu(z @ W1.T + b1)            (32, 512)
    h2   = relu(h1 @ W2.T + b2)           (32, 1024)
    flat = h2 @ W3.T + b3                 (32, 198916)   <- 815 MB of W3, the bottleneck
    per-sample 5-layer CoordinateNet on timestamps -> (32, 512, 4)

Distribution over 8 NeuronCores:
  - W3 is sharded row-wise (output-param dim) 8 ways; each core streams its
    ~102 MB shard once (memory roofline) and computes flat[:, shard].
  - flat shards are exchanged with pipelined AllToAll collectives (7 groups)
    so that core c ends up with the full param vectors of samples 4c..4c+3.
  - The CoordinateNet application is data-parallel over the batch (4/core).

Host-side prep: inputs are transposed/permuted/padded with numpy so that every
device-side DMA is a clean large-stride access (the per-sample Wh blocks of W3
are permuted to input-major so extracted weights land transposed for the PE).
"""

import os
import sys

import numpy as np

if os.path.isdir("/opt/trn_rl_repo") and "/opt/trn_rl_repo" not in sys.path:
    sys.path.insert(0, "/opt/trn_rl_repo")

import concourse.bass as bass
import concourse.mybir as mybir
import concourse.tile as tile
from concourse.bass import ts
from concourse.bass_utils import run_bass_kernel_spmd

# ---------------------------------------------------------------- constants
B = 32          # batch
NPTS = 512      # timestamps per sample
LAT = 4096      # latent dim
H1 = 512        # hyper hidden 1
H2 = 1024       # hyper hidden 2
HID = 256       # CoordinateNet hidden dim
P_TOTAL = 198916

NCORES = 8
CH = 512                  # matmul free-dim chunk
CPG = 7                   # chunks per all-to-all group
NG = 7                    # all-to-all groups
GW = CPG * CH             # 3584 columns per group
S = NG * GW               # 25088 = per-core shard of the (padded) param dim
P_PAD = NCORES * S        # 200704
BPC = B // NCORES         # 4 samples per core

WH_OFF = [512 + l * (HID * HID + HID) for l in range(3)]  # 512, 66304, 132096
WO_OFF = 197888
BO_OFF = 198912

FP = mybir.dt.float32
AF = mybir.ActivationFunctionType


# ------------------------------------------------------------- wait splitter
def _split_multi_waits(nc):
    """The walrus build here accepts at most one sync-wait per instruction.
    Engines execute in order, so hoisting all but the last wait onto fresh
    NOPs immediately before the instruction is semantically identical."""
    ctr = 0
    for f in nc.m.functions:
        for bb in f.blocks:
            out = []
            changed = False
            for ins in bb.instructions:
                si = getattr(ins, "sync_info", None)
                waits = list(si.on_wait) if (si is not None and si.on_wait) else []
                if len(waits) > 1:
                    changed = True
                    for w in waits[:-1]:
                        ctr += 1
                        out.append(
                            mybir.InstNoOp(
                                name=f"{ins.name}-sw{ctr}",
                                engine=ins.engine,
                                sync_info=mybir.SyncInfo(on_wait=[w], on_update=[]),
                            )
                        )
                    ins.sync_info = mybir.SyncInfo(
                        on_wait=waits[-1:], on_update=list(si.on_update or [])
                    )
                out.append(ins)
            if changed:
                try:
                    bb.instructions = out
                except Exception:
                    bb.instructions.clear()
                    bb.instructions.extend(out)


# ------------------------------------------------------------ device program
def _build_module(repeat: int = 1):
    nc = bass.Bass(num_devices=NCORES)

    zt_d = nc.dram_tensor("zt", [LAT, B], FP, kind="ExternalInput")
    w1t_d = nc.dram_tensor("w1t", [LAT, H1], FP, kind="ExternalInput")
    b1_d = nc.dram_tensor("b1", [H1], FP, kind="ExternalInput")
    w2t_d = nc.dram_tensor("w2t", [H1, H2], FP, kind="ExternalInput")
    b2_d = nc.dram_tensor("b2", [H2], FP, kind="ExternalInput")
    w3t_d = nc.dram_tensor("w3t", [H2 + 1, S], FP, kind="ExternalInput")
    ts_d = nc.dram_tensor("tst", [BPC, NPTS], FP, kind="ExternalInput")
    out_d = nc.dram_tensor("out", [BPC, NPTS, 4], FP, kind="ExternalOutput")

    with tile.TileContext(nc) as tc:
        with (
            tc.tile_pool(name="const", bufs=1) as const,
            tc.tile_pool(name="w1p", bufs=2) as w1p,
            tc.tile_pool(name="w3p", bufs=4) as w3p,
            tc.tile_pool(name="b3p", bufs=2) as b3p,
            tc.tile_pool(name="fsb", bufs=2) as fsb,
            tc.tile_pool(name="cpool", bufs=2) as cpool,
            tc.tile_pool(name="xpool", bufs=3) as xpool,
            tc.tile_pool(name="opool", bufs=4) as opool,
            tc.tile_pool(name="psum", bufs=8, space="PSUM") as psum,
            tc.tile_pool(name="dram", bufs=1, space="DRAM") as dram,
        ):
            for _rep in range(repeat):
                _emit_body(nc, tc, const, w1p, w3p, b3p, fsb, cpool, xpool,
                           opool, psum, dram,
                           zt_d, w1t_d, b1_d, w2t_d, b2_d, w3t_d, ts_d, out_d)

    _split_multi_waits(nc)
    return nc


def _emit_body(nc, tc, const, w1p, w3p, b3p, fsb, cpool, xpool, opool, psum,
               dram, zt_d, w1t_d, b1_d, w2t_d, b2_d, w3t_d, ts_d, out_d):
    # ---- constant loads
    zsb = const.tile([128, LAT // 128, B], FP, name="zsb", tag="zsb")
    nc.sync.dma_start(zsb[:], zt_d[:, :].rearrange("(t p) b -> p t b", p=128))
    w2sb = const.tile([128, H1 // 128, H2], FP, name="w2sb", tag="w2sb")
    nc.sync.dma_start(w2sb[:], w2t_d[:, :].rearrange("(t p) m -> p t m", p=128))
    b1sb = const.tile([128, H1 // 128], FP, name="b1sb", tag="b1sb")
    nc.sync.dma_start(b1sb[:], b1_d[:].rearrange("(t p) -> p t", p=128))
    b2sb = const.tile([128, H2 // 128], FP, name="b2sb", tag="b2sb")
    nc.sync.dma_start(b2sb[:], b2_d[:].rearrange("(t p) -> p t", p=128))
    tssb = const.tile([1, BPC, NPTS], FP, name="tssb", tag="tssb")
    nc.sync.dma_start(tssb[:], ts_d[:, :].rearrange("(a j) n -> a j n", a=1))
    ones = const.tile([1, 128], FP, name="ones", tag="ones")
    nc.gpsimd.memset(ones[:], 1.0)

    # ---- h1T = relu(W1 @ z.T + b1), stored (512, 32) as [128, 4, 32]
    h1sb = const.tile([128, 4, B], FP, name="h1sb", tag="h1sb")
    h1ps = [psum.tile([128, B], FP, name=f"h1ps{m}", tag="ps") for m in range(4)]
    for kk in range(8):
        w1sb = w1p.tile([128, 4, H1], FP, name="w1sb", tag="w1sb")
        nc.sync.dma_start(
            w1sb[:],
            w1t_d[ts(kk, 512), :].rearrange("(t p) m -> p t m", p=128),
        )
        for t4 in range(4):
            k = kk * 4 + t4
            for m in range(4):
                nc.tensor.matmul(
                    h1ps[m][:],
                    w1sb[:, t4, ts(m, 128)],
                    zsb[:, k, :],
                    start=(k == 0),
                    stop=(k == 31),
                )
    for m in range(4):
        nc.scalar.activation(
            h1sb[:, m, :], h1ps[m][:], AF.Relu, bias=b1sb[:, m : m + 1]
        )

    # ---- h2T = relu(W2 @ h1 + b2), stored (1024, 32) as [128, 8, 32]
    h2sb = const.tile([128, 8, B], FP, name="h2sb", tag="h2sb")
    for m in range(8):
        h2ps = psum.tile([128, B], FP, name="h2ps", tag="ps")
        for k in range(4):
            nc.tensor.matmul(
                h2ps[:],
                w2sb[:, k, ts(m, 128)],
                h1sb[:, k, :],
                start=(k == 0),
                stop=(k == 3),
            )
        nc.scalar.activation(
            h2sb[:, m, :], h2ps[:], AF.Relu, bias=b2sb[:, m : m + 1]
        )

    # ---- flat shard = h2 @ W3c.T + b3c, streamed in NG groups with
    #      pipelined AllToAll param exchange
    a2a_outs = []
    for g in range(NG):
        a2a_in = dram.tile([B, GW], FP, name=f"a2ain{g}", tag=f"a2ain{g}")
        a2a_out = dram.tile([B, GW], FP, name=f"a2aout{g}", tag=f"a2aout{g}")
        fps = [
            psum.tile([B, CH], FP, name=f"fps{g}_{j}", tag="ps")
            for j in range(CPG)
        ]
        for k in range(8):
            w3sb = w3p.tile([128, GW], FP, name="w3sb", tag="w3sb")
            nc.sync.dma_start(w3sb[:], w3t_d[ts(k, 128), ts(g, GW)])
            for j in range(CPG):
                nc.tensor.matmul(
                    fps[j][:],
                    h2sb[:, k, :],
                    w3sb[:, ts(j, CH)],
                    start=(k == 0),
                    stop=False,
                )
        b3row = b3p.tile([1, GW], FP, name="b3row", tag="b3row")
        nc.sync.dma_start(b3row[:], w3t_d[H2 : H2 + 1, ts(g, GW)])
        flat_sb = fsb.tile([B, GW], FP, name="flat_sb", tag="flat_sb")
        for j in range(CPG):
            nc.tensor.matmul(
                fps[j][:],
                ones[:, :B],
                b3row[:, ts(j, CH)],
                start=False,
                stop=True,
            )
            nc.vector.tensor_copy(flat_sb[:, ts(j, CH)], fps[j][:])
        nc.sync.dma_start(a2a_in[:, :], flat_sb[:])
        nc.gpsimd.collective_compute(
            "AllToAll",
            mybir.AluOpType.bypass,
            replica_groups=[list(range(NCORES))],
            ins=[a2a_in.opt()],
            outs=[a2a_out.opt()],
        )
        a2a_outs.append(a2a_out)

    # ---- assemble my 4 samples' full param vectors in DRAM
    flat4 = dram.tile([BPC, P_PAD], FP, name="flat4", tag="flat4")
    for g in range(NG):
        for j in range(BPC):
            src = a2a_outs[g].rearrange("(s r) q -> s r q", r=BPC)[:, j, :]
            dst = flat4[j].rearrange("(s c) -> s c", c=S)[:, ts(g, GW)]
            nc.sync.dma_start(dst, src)

    # ---- CoordinateNet per sample (activations feature-major:
    #      xT = [128, 2, 512] = (hid, n))
    for j in range(BPC):
        f = flat4[j]
        win = cpool.tile([1, HID], FP, name="win", tag="win")
        nc.sync.dma_start(win[:], f[0:HID].rearrange("(a b) -> a b", a=1))
        bin_ = cpool.tile([128, 2], FP, name="bin", tag="bin")
        nc.sync.dma_start(
            bin_[:], f[HID : 2 * HID].rearrange("(t p) -> p t", p=128)
        )
        xcur = xpool.tile([128, 2, NPTS], FP, name="xt", tag="xt")
        for t in range(2):
            xps = psum.tile([128, NPTS], FP, name="xps", tag="ps")
            nc.tensor.matmul(
                xps[:],
                win[:, ts(t, 128)],
                tssb[0:1, j, :],
                start=True,
                stop=True,
            )
            nc.scalar.activation(
                xcur[:, t, :], xps[:], AF.Relu, bias=bin_[:, t : t + 1]
            )
        for l in range(3):
            a = WH_OFF[l]
            wh = cpool.tile([128, 2, HID], FP, name="wh", tag="wh")
            nc.sync.dma_start(
                wh[:],
                f[a : a + HID * HID].rearrange("(t p o) -> p t o", p=128, o=HID),
            )
            bh = cpool.tile([128, 2], FP, name="bh", tag="bh")
            nc.sync.dma_start(
                bh[:],
                f[a + HID * HID : a + HID * HID + HID].rearrange(
                    "(t p) -> p t", p=128
                ),
            )
            xnext = xpool.tile([128, 2, NPTS], FP, name="xt", tag="xt")
            for m in range(2):
                hps = psum.tile([128, NPTS], FP, name="hps", tag="ps")
                for t in range(2):
                    nc.tensor.matmul(
                        hps[:],
                        wh[:, t, ts(m, 128)],
                        xcur[:, t, :],
                        start=(t == 0),
                        stop=(t == 1),
                    )
                nc.scalar.activation(
                    xnext[:, m, :], hps[:], AF.Relu, bias=bh[:, m : m + 1]
                )
            xcur = xnext
        # output layer, n-major so the store is contiguous
        wo = cpool.tile([128, 2, 4], FP, name="wo", tag="wo")
        nc.sync.dma_start(
            wo[:],
            f[WO_OFF : WO_OFF + 4 * HID].rearrange("(t p o) -> p t o", p=128, o=4),
        )
        bo = cpool.tile([1, 4], FP, name="bo", tag="bo")
        nc.sync.dma_start(
            bo[:], f[BO_OFF : BO_OFF + 4].rearrange("(a b) -> a b", a=1)
        )
        for m in range(4):
            ops_ = psum.tile([128, 4], FP, name="ops", tag="ps")
            for t in range(2):
                nc.tensor.matmul(
                    ops_[:],
                    xcur[:, t, ts(m, 128)],
                    wo[:, t, :],
                    start=(t == 0),
                    stop=False,
                )
            nc.tensor.matmul(
                ops_[:], ones[:, :128], bo[:], start=False, stop=True
            )
            outm = opool.tile([128, 4], FP, name="outm", tag="outm")
            nc.scalar.activation(outm[:], ops_[:], AF.Sigmoid)
            nc.sync.dma_start(out_d[j, ts(m, 128), :], outm[:])


_NC_CACHE = {}


def _get_module(repeat: int = 1):
    if repeat not in _NC_CACHE:
        _NC_CACHE[repeat] = _build_module(repeat)
    return _NC_CACHE[repeat]


# -------------------------------------------------------------- host wrapper
def _build_perm():
    perm = np.arange(P_TOTAL, dtype=np.int64)
    g = np.arange(HID * HID, dtype=np.int64).reshape(HID, HID)
    for a in WH_OFF:
        perm[a : a + HID * HID] = a + g.T.ravel()
    g2 = np.arange(4 * HID, dtype=np.int64).reshape(4, HID)
    perm[WO_OFF : WO_OFF + 4 * HID] = WO_OFF + g2.T.ravel()
    return perm


_PERM_CACHE = None
LAST_RESULTS = None


def prepare_in_maps(z, timestamps, W1, b1, W2, b2, W3, b3):
    global _PERM_CACHE
    z = np.asarray(z, np.float32)
    timestamps = np.asarray(timestamps, np.float32)
    W1 = np.asarray(W1, np.float32)
    b1 = np.asarray(b1, np.float32)
    W2 = np.asarray(W2, np.float32)
    b2 = np.asarray(b2, np.float32)
    W3 = np.asarray(W3, np.float32)
    b3 = np.asarray(b3, np.float32)

    if _PERM_CACHE is None:
        _PERM_CACHE = _build_perm()
    perm = _PERM_CACHE

    zt = np.ascontiguousarray(z.T)
    w1t = np.ascontiguousarray(W1.T)
    w2t = np.ascontiguousarray(W2.T)
    Wp = W3[perm]        # rows permuted to extraction-friendly order
    bp = b3[perm]

    in_maps = []
    for c in range(NCORES):
        lo = c * S
        n = max(0, min((c + 1) * S, P_TOTAL) - lo)
        w3t_c = np.zeros((H2 + 1, S), np.float32)
        if n > 0:
            w3t_c[:H2, :n] = Wp[lo : lo + n].T
            w3t_c[H2, :n] = bp[lo : lo + n]
        in_maps.append(
            {
                "zt": zt,
                "w1t": w1t,
                "b1": b1,
                "w2t": w2t,
                "b2": b2,
                "w3t": w3t_c,
                "tst": np.ascontiguousarray(
                    timestamps[c * BPC : (c + 1) * BPC, :, 0]
                ),
            }
        )
    return in_maps


def kernel(z, timestamps, W1, b1, W2, b2, W3, b3):
    global LAST_RESULTS
    in_maps = prepare_in_maps(z, timestamps, W1, b1, W2, b2, W3, b3)
    nc = _get_module(1)
    res = run_bass_kernel_spmd(nc, in_maps, core_ids=list(range(NCORES)))
    LAST_RESULTS = res
    out = np.concatenate(
        [np.asarray(res.results[c]["out"]) for c in range(NCORES)], axis=0
    )
    return out.astype(np.float32, copy=False)


# revision 6
# speedup vs baseline: 19.8830x; 19.8830x over previous
"""Trainium2 Bass kernel for nn_BBoxDecoder (HyperNetwork -> per-sample CoordinateNet).

Computation (fp32 accuracy):
    h1   = relu(z @ W1.T + b1)            (32, 512)
    h2   = relu(h1 @ W2.T + b2)           (32, 1024)
    flat = h2 @ W3.T + b3                 (32, 198916)   <- 815 MB of W3, the bottleneck
    per-sample 5-layer CoordinateNet on timestamps -> (32, 512, 4)

Distribution over 8 NeuronCores:
  - W3 is sharded row-wise (output-param dim) 8 ways; each core streams its
    ~102 MB shard once (memory roofline) and computes flat[:, shard].
  - flat shards are exchanged with pipelined AllToAll collectives (7 groups)
    so that core c ends up with the full param vectors of samples 4c..4c+3.
  - The CoordinateNet application is data-parallel over the batch (4/core).

The big matmul streams W3 as an fp16 hi/lo pair (same 4 bytes/element as fp32)
and runs 3 fp16 passes per tile (hh, lh, hl). fp16 pairs carry 22 mantissa
bits, so the result matches fp32 matmul precision while the PE runs at 3
cycles/row instead of fp32's 4. Everything is pre-scaled by 32 (h2) / 32 (W3)
on host so the fp16 lo-planes stay in normal range; the PSUM->SBUF copy
divides by 1024 (exact power of two).

Host-side prep: inputs are transposed/permuted/padded with numpy so that every
device-side DMA is a clean large-stride access (the per-sample Wh blocks of W3
are permuted to input-major so extracted weights land transposed for the PE).
"""

import os
import sys

import numpy as np

if os.path.isdir("/opt/trn_rl_repo") and "/opt/trn_rl_repo" not in sys.path:
    sys.path.insert(0, "/opt/trn_rl_repo")

import concourse.bass as bass
import concourse.mybir as mybir
import concourse.tile as tile
from concourse.bass import ts
from concourse.bass_utils import run_bass_kernel_spmd

# ---------------------------------------------------------------- constants
B = 32          # batch
NPTS = 512      # timestamps per sample
LAT = 4096      # latent dim
H1 = 512        # hyper hidden 1
H2 = 1024       # hyper hidden 2
HID = 256       # CoordinateNet hidden dim
P_TOTAL = 198916

NCORES = 8
CH = 512                  # matmul free-dim chunk
CPG = 7                   # chunks per all-to-all group
NG = 7                    # all-to-all groups
GW = CPG * CH             # 3584 columns per group
S = NG * GW               # 25088 = per-core shard of the (padded) param dim
P_PAD = NCORES * S        # 200704
BPC = B // NCORES         # 4 samples per core

WH_OFF = [512 + l * (HID * HID + HID) for l in range(3)]  # 512, 66304, 132096
WO_OFF = 197888
BO_OFF = 198912

SCALE = 32.0              # h2 and W3 pre-scale; flat comes out x1024

FP = mybir.dt.float32
F16 = mybir.dt.float16
AF = mybir.ActivationFunctionType


# ------------------------------------------------------------- wait splitter
def _split_multi_waits(nc):
    """The walrus build here accepts at most one sync-wait per instruction.
    Engines execute in order, so hoisting all but the last wait onto fresh
    NOPs immediately before the instruction is semantically identical."""
    ctr = 0
    for f in nc.m.functions:
        for bb in f.blocks:
            out = []
            changed = False
            for ins in bb.instructions:
                si = getattr(ins, "sync_info", None)
                waits = list(si.on_wait) if (si is not None and si.on_wait) else []
                if len(waits) > 1:
                    changed = True
                    for w in waits[:-1]:
                        ctr += 1
                        out.append(
                            mybir.InstNoOp(
                                name=f"{ins.name}-sw{ctr}",
                                engine=ins.engine,
                                sync_info=mybir.SyncInfo(on_wait=[w], on_update=[]),
                            )
                        )
                    ins.sync_info = mybir.SyncInfo(
                        on_wait=waits[-1:], on_update=list(si.on_update or [])
                    )
                out.append(ins)
            if changed:
                try:
                    bb.instructions = out
                except Exception:
                    bb.instructions.clear()
                    bb.instructions.extend(out)


# ------------------------------------------------------------ device program
def _build_module(repeat: int = 1):
    nc = bass.Bass(num_devices=NCORES)

    zt_d = nc.dram_tensor("zt", [LAT, B], FP, kind="ExternalInput")
    w1t_d = nc.dram_tensor("w1t", [LAT, H1], FP, kind="ExternalInput")
    b1_d = nc.dram_tensor("b1", [H1], FP, kind="ExternalInput")
    w2t_d = nc.dram_tensor("w2t", [H1, H2], FP, kind="ExternalInput")
    b2_d = nc.dram_tensor("b2s", [H2], FP, kind="ExternalInput")     # 32*b2
    w3h_d = nc.dram_tensor("w3h", [H2 + 1, S], F16, kind="ExternalInput")
    w3l_d = nc.dram_tensor("w3l", [H2 + 1, S], F16, kind="ExternalInput")
    ts_d = nc.dram_tensor("tst", [BPC, NPTS], FP, kind="ExternalInput")
    out_d = nc.dram_tensor("out", [BPC, NPTS, 4], FP, kind="ExternalOutput")

    with tile.TileContext(nc) as tc:
        with (
            tc.tile_pool(name="const", bufs=1) as const,
            tc.tile_pool(name="w1p", bufs=2) as w1p,
            tc.tile_pool(name="w3hp", bufs=4) as w3hp,
            tc.tile_pool(name="w3lp", bufs=4) as w3lp,
            tc.tile_pool(name="b3p", bufs=2) as b3p,
            tc.tile_pool(name="fsb", bufs=2) as fsb,
            tc.tile_pool(name="cpool", bufs=2) as cpool,
            tc.tile_pool(name="xpool", bufs=3) as xpool,
            tc.tile_pool(name="opool", bufs=4) as opool,
            tc.tile_pool(name="psum", bufs=8, space="PSUM") as psum,
            tc.tile_pool(name="dram", bufs=1, space="DRAM") as dram,
        ):
            for _rep in range(repeat):
                _emit_body(nc, tc, const, w1p, w3hp, w3lp, b3p, fsb, cpool,
                           xpool, opool, psum, dram,
                           zt_d, w1t_d, b1_d, w2t_d, b2_d, w3h_d, w3l_d,
                           ts_d, out_d)

    _split_multi_waits(nc)
    return nc


def _emit_body(nc, tc, const, w1p, w3hp, w3lp, b3p, fsb, cpool, xpool, opool,
               psum, dram, zt_d, w1t_d, b1_d, w2t_d, b2_d, w3h_d, w3l_d,
               ts_d, out_d):
    # ---- constant loads
    zsb = const.tile([128, LAT // 128, B], FP, name="zsb", tag="zsb")
    nc.sync.dma_start(zsb[:], zt_d[:, :].rearrange("(t p) b -> p t b", p=128))
    w2sb = const.tile([128, H1 // 128, H2], FP, name="w2sb", tag="w2sb")
    nc.sync.dma_start(w2sb[:], w2t_d[:, :].rearrange("(t p) m -> p t m", p=128))
    b1sb = const.tile([128, H1 // 128], FP, name="b1sb", tag="b1sb")
    nc.sync.dma_start(b1sb[:], b1_d[:].rearrange("(t p) -> p t", p=128))
    b2sb = const.tile([128, H2 // 128], FP, name="b2sb", tag="b2sb")
    nc.sync.dma_start(b2sb[:], b2_d[:].rearrange("(t p) -> p t", p=128))
    tssb = const.tile([1, BPC, NPTS], FP, name="tssb", tag="tssb")
    nc.sync.dma_start(tssb[:], ts_d[:, :].rearrange("(a j) n -> a j n", a=1))
    ones = const.tile([1, 128], FP, name="ones", tag="ones")
    nc.gpsimd.memset(ones[:], 1.0)
    ones16 = const.tile([1, B], F16, name="ones16", tag="ones16")
    nc.gpsimd.memset(ones16[:], 1.0)

    # ---- h1T = relu(W1 @ z.T + b1), stored (512, 32) as [128, 4, 32]
    h1sb = const.tile([128, 4, B], FP, name="h1sb", tag="h1sb")
    h1ps = [psum.tile([128, B], FP, name=f"h1ps{m}", tag="ps") for m in range(4)]
    for kk in range(8):
        w1sb = w1p.tile([128, 4, H1], FP, name="w1sb", tag="w1sb")
        nc.sync.dma_start(
            w1sb[:],
            w1t_d[ts(kk, 512), :].rearrange("(t p) m -> p t m", p=128),
        )
        for t4 in range(4):
            k = kk * 4 + t4
            for m in range(4):
                nc.tensor.matmul(
                    h1ps[m][:],
                    w1sb[:, t4, ts(m, 128)],
                    zsb[:, k, :],
                    start=(k == 0),
                    stop=(k == 31),
                )
    for m in range(4):
        nc.scalar.activation(
            h1sb[:, m, :], h1ps[m][:], AF.Relu, bias=b1sb[:, m : m + 1]
        )

    # ---- h2s = 32*relu(W2 @ h1 + b2) as fp16 hi/lo pair [128, 8, 32]
    h2h = const.tile([128, 8, B], F16, name="h2h", tag="h2h")
    h2l = const.tile([128, 8, B], F16, name="h2l", tag="h2l")
    h2f = const.tile([128, 8, B], FP, name="h2f", tag="h2f")
    for m in range(8):
        h2ps = psum.tile([128, B], FP, name="h2ps", tag="ps")
        for k in range(4):
            nc.tensor.matmul(
                h2ps[:],
                w2sb[:, k, ts(m, 128)],
                h1sb[:, k, :],
                start=(k == 0),
                stop=(k == 3),
            )
        # 32*relu(x + b2) == relu(32x + 32*b2); b2s is pre-scaled on host
        nc.scalar.activation(
            h2f[:, m, :], h2ps[:], AF.Relu, bias=b2sb[:, m : m + 1], scale=SCALE
        )
        nc.vector.tensor_copy(h2h[:, m, :], h2f[:, m, :])
        nc.vector.tensor_sub(h2l[:, m, :], h2f[:, m, :], h2h[:, m, :])

    # ---- flat shard = h2 @ W3c.T + b3c (x1024, fp16-pair passes), streamed
    #      in NG groups with pipelined AllToAll param exchange
    a2a_outs = []
    for g in range(NG):
        a2a_in = dram.tile([B, GW], FP, name=f"a2ain{g}", tag=f"a2ain{g}")
        a2a_out = dram.tile([B, GW], FP, name=f"a2aout{g}", tag=f"a2aout{g}")
        fps = [
            psum.tile([B, CH], FP, name=f"fps{g}_{j}", tag="ps")
            for j in range(CPG)
        ]
        for k in range(8):
            w3hsb = w3hp.tile([128, GW], F16, name="w3hsb", tag="w3hsb")
            nc.sync.dma_start(w3hsb[:], w3h_d[ts(k, 128), ts(g, GW)])
            w3lsb = w3lp.tile([128, GW], F16, name="w3lsb", tag="w3lsb")
            nc.sync.dma_start(w3lsb[:], w3l_d[ts(k, 128), ts(g, GW)])
            for j in range(CPG):
                nc.tensor.matmul(
                    fps[j][:],
                    h2h[:, k, :],
                    w3hsb[:, ts(j, CH)],
                    start=(k == 0),
                    stop=False,
                )
                nc.tensor.matmul(
                    fps[j][:],
                    h2l[:, k, :],
                    w3hsb[:, ts(j, CH)],
                    start=False,
                    stop=False,
                )
                nc.tensor.matmul(
                    fps[j][:],
                    h2h[:, k, :],
                    w3lsb[:, ts(j, CH)],
                    start=False,
                    stop=False,
                )
        b3rh = b3p.tile([1, GW], F16, name="b3rh", tag="b3rh")
        nc.sync.dma_start(b3rh[:], w3h_d[H2 : H2 + 1, ts(g, GW)])
        b3rl = b3p.tile([1, GW], F16, name="b3rl", tag="b3rl")
        nc.sync.dma_start(b3rl[:], w3l_d[H2 : H2 + 1, ts(g, GW)])
        flat_sb = fsb.tile([B, GW], FP, name="flat_sb", tag="flat_sb")
        for j in range(CPG):
            nc.tensor.matmul(
                fps[j][:], ones16[:], b3rh[:, ts(j, CH)], start=False, stop=False
            )
            nc.tensor.matmul(
                fps[j][:], ones16[:], b3rl[:, ts(j, CH)], start=False, stop=True
            )
            # undo the 32*32 pre-scale (exact power of two)
            nc.scalar.mul(flat_sb[:, ts(j, CH)], fps[j][:], 1.0 / 1024.0)
        nc.sync.dma_start(a2a_in[:, :], flat_sb[:])
        nc.gpsimd.collective_compute(
            "AllToAll",
            mybir.AluOpType.bypass,
            replica_groups=[list(range(NCORES))],
            ins=[a2a_in.opt()],
            outs=[a2a_out.opt()],
        )
        a2a_outs.append(a2a_out)

    # ---- assemble my 4 samples' full param vectors in DRAM
    flat4 = dram.tile([BPC, P_PAD], FP, name="flat4", tag="flat4")
    for g in range(NG):
        for j in range(BPC):
            src = a2a_outs[g].rearrange("(s r) q -> s r q", r=BPC)[:, j, :]
            dst = flat4[j].rearrange("(s c) -> s c", c=S)[:, ts(g, GW)]
            nc.sync.dma_start(dst, src)

    # ---- CoordinateNet per sample (activations feature-major:
    #      xT = [128, 2, 512] = (hid, n))
    for j in range(BPC):
        f = flat4[j]
        win = cpool.tile([1, HID], FP, name="win", tag="win")
        nc.sync.dma_start(win[:], f[0:HID].rearrange("(a b) -> a b", a=1))
        bin_ = cpool.tile([128, 2], FP, name="bin", tag="bin")
        nc.sync.dma_start(
            bin_[:], f[HID : 2 * HID].rearrange("(t p) -> p t", p=128)
        )
        xcur = xpool.tile([128, 2, NPTS], FP, name="xt", tag="xt")
        for t in range(2):
            xps = psum.tile([128, NPTS], FP, name="xps", tag="ps")
            nc.tensor.matmul(
                xps[:],
                win[:, ts(t, 128)],
                tssb[0:1, j, :],
                start=True,
                stop=True,
            )
            nc.scalar.activation(
                xcur[:, t, :], xps[:], AF.Relu, bias=bin_[:, t : t + 1]
            )
        for l in range(3):
            a = WH_OFF[l]
            wh = cpool.tile([128, 2, HID], FP, name="wh", tag="wh")
            nc.sync.dma_start(
                wh[:],
                f[a : a + HID * HID].rearrange("(t p o) -> p t o", p=128, o=HID),
            )
            bh = cpool.tile([128, 2], FP, name="bh", tag="bh")
            nc.sync.dma_start(
                bh[:],
                f[a + HID * HID : a + HID * HID + HID].rearrange(
                    "(t p) -> p t", p=128
                ),
            )
            xnext = xpool.tile([128, 2, NPTS], FP, name="xt", tag="xt")
            for m in range(2):
                hps = psum.tile([128, NPTS], FP, name="hps", tag="ps")
                for t in range(2):
                    nc.tensor.matmul(
                        hps[:],
                        wh[:, t, ts(m, 128)],
                        xcur[:, t, :],
                        start=(t == 0),
                        stop=(t == 1),
                    )
                nc.scalar.activation(
                    xnext[:, m, :], hps[:], AF.Relu, bias=bh[:, m : m + 1]
                )
            xcur = xnext
        # output layer, n-major so the store is contiguous
        wo = cpool.tile([128, 2, 4], FP, name="wo", tag="wo")
        nc.sync.dma_start(
            wo[:],
            f[WO_OFF : WO_OFF + 4 * HID].rearrange("(t p o) -> p t o", p=128, o=4),
        )
        bo = cpool.tile([1, 4], FP, name="bo", tag="bo")
        nc.sync.dma_start(
            bo[:], f[BO_OFF : BO_OFF + 4].rearrange("(a b) -> a b", a=1)
        )
        for m in range(4):
            ops_ = psum.tile([128, 4], FP, name="ops", tag="ps")
            for t in range(2):
                nc.tensor.matmul(
                    ops_[:],
                    xcur[:, t, ts(m, 128)],
                    wo[:, t, :],
                    start=(t == 0),
                    stop=False,
                )
            nc.tensor.matmul(
                ops_[:], ones[:, :128], bo[:], start=False, stop=True
            )
            outm = opool.tile([128, 4], FP, name="outm", tag="outm")
            nc.scalar.activation(outm[:], ops_[:], AF.Sigmoid)
            nc.sync.dma_start(out_d[j, ts(m, 128), :], outm[:])


_NC_CACHE = {}


def _get_module(repeat: int = 1):
    if repeat not in _NC_CACHE:
        _NC_CACHE[repeat] = _build_module(repeat)
    return _NC_CACHE[repeat]


# -------------------------------------------------------------- host wrapper
def _build_perm():
    perm = np.arange(P_TOTAL, dtype=np.int64)
    g = np.arange(HID * HID, dtype=np.int64).reshape(HID, HID)
    for a in WH_OFF:
        perm[a : a + HID * HID] = a + g.T.ravel()
    g2 = np.arange(4 * HID, dtype=np.int64).reshape(4, HID)
    perm[WO_OFF : WO_OFF + 4 * HID] = WO_OFF + g2.T.ravel()
    return perm


_PERM_CACHE = None
LAST_RESULTS = None


def prepare_in_maps(z, timestamps, W1, b1, W2, b2, W3, b3):
    global _PERM_CACHE
    z = np.asarray(z, np.float32)
    timestamps = np.asarray(timestamps, np.float32)
    W1 = np.asarray(W1, np.float32)
    b1 = np.asarray(b1, np.float32)
    W2 = np.asarray(W2, np.float32)
    b2 = np.asarray(b2, np.float32)
    W3 = np.asarray(W3, np.float32)
    b3 = np.asarray(b3, np.float32)

    if _PERM_CACHE is None:
        _PERM_CACHE = _build_perm()
    perm = _PERM_CACHE

    zt = np.ascontiguousarray(z.T)
    w1t = np.ascontiguousarray(W1.T)
    w2t = np.ascontiguousarray(W2.T)
    b2s = 32.0 * b2
    Wp = W3[perm]        # rows permuted to extraction-friendly order
    bp = b3[perm]

    in_maps = []
    for c in range(NCORES):
        lo = c * S
        n = max(0, min((c + 1) * S, P_TOTAL) - lo)
        w3h_c = np.zeros((H2 + 1, S), np.float16)
        w3l_c = np.zeros((H2 + 1, S), np.float16)
        if n > 0:
            ws = 32.0 * Wp[lo : lo + n]                          # (n, 1024) f32
            hi = ws.astype(np.float16)
            lo_plane = (ws - hi.astype(np.float32)).astype(np.float16)
            w3h_c[:H2, :n] = hi.T
            w3l_c[:H2, :n] = lo_plane.T
            bs = 1024.0 * bp[lo : lo + n]
            bhi = bs.astype(np.float16)
            w3h_c[H2, :n] = bhi
            w3l_c[H2, :n] = (bs - bhi.astype(np.float32)).astype(np.float16)
        in_maps.append(
            {
                "zt": zt,
                "w1t": w1t,
                "b1": b1,
                "w2t": w2t,
                "b2s": b2s,
                "w3h": w3h_c,
                "w3l": w3l_c,
                "tst": np.ascontiguousarray(
                    timestamps[c * BPC : (c + 1) * BPC, :, 0]
                ),
            }
        )
    return in_maps


def kernel(z, timestamps, W1, b1, W2, b2, W3, b3):
    global LAST_RESULTS
    in_maps = prepare_in_maps(z, timestamps, W1, b1, W2, b2, W3, b3)
    nc = _get_module(1)
    res = run_bass_kernel_spmd(nc, in_maps, core_ids=list(range(NCORES)))
    LAST_RESULTS = res
    out = np.concatenate(
        [np.asarray(res.results[c]["out"]) for c in range(NCORES)], axis=0
    )
    return out.astype(np.float32, copy=False)
